# revision 6
# baseline (speedup 1.0000x reference)
"""Trainium2 Bass kernel for DisentangleStaticNoiseLoss (NT-Xent style loss).

Math (matches the jax reference):
    x   : [K=8192, D=128] stacked embeddings (N=8 blocks of BS=1024)
    z   : row-normalized x;  S = (z @ z.T) / 0.5;  E = exp(S)
    row i (block b, sample r): positives = S[i, r + b'*BS] for b' != b,
    negatives = all j with j % BS != r.
    loss = mean over (i, pos) of [log(exp(pos) + sum_neg exp(neg)) - pos]

Sharding (data-parallel over rows, symmetric halving of the exp work):
E is symmetric, so the K x K matrix is computed only once per unordered
block pair instead of twice.  Each core receives the FULL z rotated so its
own 1024 rows come first (host-side np.roll), making the SPMD program
identical on every core.  Core c computes, for its rows, the local column
blocks B0..B3 fully plus the upper-triangular half of B4 (ragged by
128-row m-tile).  Block-pair coverage over all 8 cores:
  - difference d=1,2,3 pairs: covered once via B1..B3,
  - d=5,6,7 pairs: same unordered pairs, covered from the other side,
  - d=4 pairs: each of the two cores computes one triangle of the pair
    block; the 128x128 pair-diagonal subtiles are computed fully by BOTH
    cores, and a per-row correction column (their full row-sum) removes
    the double count at assembly time.
Per core the device emits:
  - rows_out [128,24] f32: per-(m-tile, slice) row sums of E (ACT accum),
  - cols_out [8,512]  f32: column sums of E over the core's rows for local
    columns 1024..5119 (PE ones-matmul into one PSUM bank, chunk k in
    PSUM partition k, accumulated across m-tiles),
  - g_out [128,48] f32: per-m-tile diagonals of B0..B4 (the positive-pair
    exp values + the self term) and the B4 double-count correction.
The host (the gather/unshard step) sums the row/col contributions into
F_i = sum_j E_ij, forms A_i = F_i - sum_b exp(pos), and reduces the
final scalar loss in float64: mean(log(exp(p)+A) - p).  This is the
all-reduce of the sharded partial sums; an on-device NRT collective
costs ~15-28us constant overhead, more than the entire saving.

Engine budget per core: ACT exps 128x37376 elements in 24 slices
(~49us, the bottleneck), PE ~28us (S matmuls + colsum matmuls), DVE
~10us (diag extracts + corr), one Pool copy, DMA 2MB in / 44KB out.
"""

import sys

import numpy as np

if "/opt/trn_rl_repo" not in sys.path:
    sys.path.insert(0, "/opt/trn_rl_repo")

N = 8
BS = 1024
D = 128
K = N * BS          # 8192
NCORES = 8
TEMP_SCALE = 2.0    # 1 / temperature

_NC_CACHE = {}


def _slice_sizes(m):
    """Per-m-tile activation slice widths over the 4096+w column stream."""
    w = 1024 - 128 * m
    total = 4096 + w
    if m % 2 == 0:
        sl = [2048, 1536, total - 3584]
    else:
        sl = [1536, 2048, total - 3584]
    assert sl[2] >= 128 and sl[2] <= 2048
    return sl


def _build_nc():
    import concourse.bacc as bacc
    import concourse.tile as tile
    from concourse import mybir

    f32 = mybir.dt.float32
    bf16 = mybir.dt.bfloat16
    AX = mybir.AxisListType
    OP = mybir.AluOpType
    AF = mybir.ActivationFunctionType

    nc = bacc.Bacc("TRN2", target_bir_lowering=False, debug=False)
    zt = nc.declare_dram_parameter("zt", [128, K], bf16, isOutput=False)
    ident = nc.declare_dram_parameter("ident", [128, 128], bf16, isOutput=False)
    # ind[:, 8k:8k+8] is the [128,8] indicator matrix with column k all-ones:
    # used as matmul lhsT it lands chunk k's column sums in PSUM partition k
    # (and adds zeros to the other partitions of the shared strip tile).
    ind = nc.declare_dram_parameter("ind", [128, 64], bf16, isOutput=False)
    rows_out = nc.declare_dram_parameter("rows_out", [128, 24], f32, isOutput=True)
    g_out = nc.declare_dram_parameter("g_out", [128, 48], f32, isOutput=True)
    cols_out = nc.declare_dram_parameter("cols_out", [8, 512], f32, isOutput=True)

    with tile.TileContext(nc) as tc:
        with (
            tc.tile_pool(name="persist", bufs=1) as P,
            tc.tile_pool(name="work", bufs=2) as W,
            tc.tile_pool(name="pmm", bufs=1, space="PSUM") as PM,
        ):
            # persistent SBUF
            ztg = []
            for g in range(8):
                zg = P.tile([128, 1024], bf16, tag=f"ztg{g}")
                ztg.append(zg)
                nc.gpsimd.dma_start(out=zg[:], in_=zt[:, g * 1024 : (g + 1) * 1024])
            idsb = P.tile([128, 128], bf16, tag="idsb")
            nc.sync.dma_start(out=idsb[:], in_=ident[:, :])
            indsb = P.tile([128, 64], bf16, tag="indsb")
            nc.sync.dma_start(out=indsb[:], in_=ind[:, :])
            rows_acc = P.tile([128, 24], f32, tag="rows_acc")
            g_acc = P.tile([128, 48], f32, tag="g_acc")

            # PSUM: two S tiles (double buffer) + colsum strip bank
            sA = PM.tile([128, 2048], f32, tag="sA")
            sB = PM.tile([128, 1536], f32, tag="sB")
            CS = PM.tile([128, 512], f32, tag="cs")

            def rhs_ap(c0, c1):
                """zt SBUF AP for local column range [c0, c1) (single group)."""
                g = c0 // 1024
                assert (c1 - 1) // 1024 == g
                return ztg[g][:, c0 - g * 1024 : c1 - g * 1024]

            for m in range(8):
                w = 1024 - 128 * m
                lhsT = ztg[0][:, m * 128 : (m + 1) * 128]
                E = W.tile([128, 5120], bf16, tag="E")
                sl = _slice_sizes(m)
                so = 0  # stream offset
                for si, L in enumerate(sl):
                    ps = sA if (si + m) % 2 == 0 else sB
                    # fill PSUM with S via 512-col matmul chunks
                    q0 = 0
                    while q0 < L:
                        qw = min(512, L - q0)
                        s_pos = so + q0
                        # local col of stream position
                        if s_pos < 4096:
                            c0 = s_pos
                        else:
                            c0 = s_pos + 128 * m
                        nc.tensor.matmul(
                            ps[:, q0 : q0 + qw],
                            lhsT,
                            rhs_ap(c0, c0 + qw),
                            start=True,
                            stop=True,
                        )
                        q0 += qw
                    nc.scalar.activation(
                        out=E[:, so : so + L],
                        in_=ps[:, 0:L],
                        func=AF.Exp,
                        scale=TEMP_SCALE,
                        accum_out=rows_acc[:, 3 * m + si : 3 * m + si + 1],
                    )
                    so += L

                # diag extracts: positives + self term (stream pos k*1024+m*128
                # for k<4; B4 diag subtile sits at stream 4096)
                for k in range(5):
                    cs0 = k * 1024 + m * 128 if k < 4 else 4096
                    scr = W.tile([128, 128], bf16, tag="scr")
                    nc.vector.scalar_tensor_tensor(
                        out=scr[:],
                        in0=E[:, cs0 : cs0 + 128],
                        scalar=1.0,
                        in1=idsb[:],
                        op0=OP.mult,
                        op1=OP.mult,
                        accum_out=g_acc[:, 6 * m + k : 6 * m + k + 1],
                    )
                # corr: full row-sum of the B4 pair-diagonal subtile
                nc.vector.tensor_reduce(
                    out=g_acc[:, 6 * m + 5 : 6 * m + 6],
                    in_=E[:, 4096 : 4096 + 128],
                    axis=AX.X,
                    op=OP.add,
                )

                # colsum matmuls: local cols 1024..4096+w, chunk k of 512 cols
                # accumulated across m into PSUM partition k of CS via the
                # indicator lhsT (adds zeros to the other partitions)
                for k in range(8):
                    c0 = 1024 + 512 * k
                    c1 = c0 + 512
                    if k < 6:
                        j0, j1 = 0, 512
                        st0 = c0
                    else:
                        # B4 chunks: valid cols are >= 4096+128m
                        lo = max(c0, 4096 + 128 * m)
                        if lo >= c1:
                            continue
                        j0, j1 = lo - c0, 512
                        st0 = lo - 128 * m  # stream position
                    nc.tensor.matmul(
                        CS[0:8, j0:j1],
                        indsb[:, 8 * k : 8 * k + 8],
                        E[:, st0 : st0 + (j1 - j0)],
                        start=(m == 0 and k == 0),
                        stop=(m == 7 and k == 7),
                        skip_group_check=True,
                    )

            # drain CS -> SBUF -> DRAM; ship accumulators
            cs_sb = P.tile([8, 512], f32, tag="cs_sb")
            nc.vector.tensor_copy(out=cs_sb[:], in_=CS[0:8, :])
            nc.sync.dma_start(out=cols_out[:, :], in_=cs_sb[:])
            nc.sync.dma_start(out=rows_out[:, :], in_=rows_acc[:])
            nc.sync.dma_start(out=g_out[:, :], in_=g_acc[:])

    nc.compile()
    return nc


def _get_nc():
    if "nc" not in _NC_CACHE:
        _NC_CACHE["nc"] = _build_nc()
    return _NC_CACHE["nc"]


def _host_prep(sim):
    import ml_dtypes

    x = np.asarray(sim, dtype=np.float64).reshape(K, D)
    z = (x / np.maximum(np.linalg.norm(x, axis=1, keepdims=True), 1e-8)).astype(
        np.float32
    )
    ident = np.eye(128, dtype=ml_dtypes.bfloat16)
    ind = np.zeros((128, 64), dtype=ml_dtypes.bfloat16)
    for k in range(8):
        ind[:, 8 * k + k] = 1
    in_maps = []
    for c in range(NCORES):
        ztc = np.ascontiguousarray(
            np.roll(z, -c * BS, axis=0).T.astype(ml_dtypes.bfloat16)
        )
        in_maps.append({"zt": ztc, "ident": ident, "ind": ind})
    return in_maps


def _assemble(results):
    """Gather/unshard: combine per-core partial sums into the scalar loss."""
    F = np.zeros(K, np.float64)
    G = np.zeros((K, 8), np.float64)
    li = np.arange(128)
    for c in range(NCORES):
        rows = np.asarray(results[c]["rows_out"], np.float64)
        g = np.asarray(results[c]["g_out"], np.float64)
        cols = np.asarray(results[c]["cols_out"], np.float64)
        for m in range(8):
            gr = c * BS + m * 128 + li  # global rows
            F[gr] += rows[:, 3 * m : 3 * m + 3].sum(1)
            F[gr] -= g[:, 6 * m + 5]  # B4 diag-subtile double count
            for k in range(5):
                bc = (c + k) % 8
                G[gr, bc] = g[:, 6 * m + k]
                G[bc * BS + m * 128 + li, c] = g[:, 6 * m + k]
        for k in range(8):
            gcols = (c * BS + 1024 + 512 * k + np.arange(512)) % K
            F[gcols] += cols[k]
    P = G.sum(1)
    A = F - P
    idx = np.arange(K)
    mask = np.ones((K, 8), bool)
    mask[idx, idx // BS] = False
    Epos = G[mask].reshape(K, 7)
    L = np.log(Epos + A[:, None]) - np.log(Epos)
    return np.float32(L.sum() / (K * 7))


def kernel(sim: np.ndarray, _want_results: bool = False, _trace: bool = False):
    in_maps = _host_prep(sim)
    nc = _get_nc()
    from concourse.bass_utils import run_bass_kernel_spmd

    res = run_bass_kernel_spmd(nc, in_maps, list(range(NCORES)), trace=_trace)
    loss = _assemble(res.results)
    if _want_results:
        return loss, res
    return loss


if __name__ == "__main__":
    nc = _build_nc()
    print("build OK")


# revision 10
# speedup vs baseline: 1.0915x; 1.0915x over previous
"""Trainium2 Bass kernel for DisentangleStaticNoiseLoss (NT-Xent style loss).

Math (matches the jax reference):
    x   : [K=8192, D=128] stacked embeddings (N=8 blocks of BS=1024)
    z   : row-normalized x;  S = (z @ z.T) / 0.5;  E = exp(S)
    row i (block b, sample r): positives = S[i, r + b'*BS] for b' != b,
    negatives = all j with j % BS != r.
    loss = mean over (i, pos) of [log(exp(pos) + sum_neg exp(neg)) - pos]

Sharding (data-parallel over rows, symmetric halving of the exp work):
E is symmetric, so the K x K matrix is computed only once per unordered
block pair instead of twice.  Each core receives the FULL z rotated so its
own 1024 rows come first (host-side np.roll), making the SPMD program
identical on every core.  Core c computes, for its rows, the local column
blocks B0..B3 fully plus the upper-triangular half of B4 (ragged by
128-row m-tile).  Block-pair coverage over all 8 cores:
  - difference d=1,2,3 pairs: covered once via B1..B3,
  - d=5,6,7 pairs: same unordered pairs, covered from the other side,
  - d=4 pairs: each of the two cores computes one triangle of the pair
    block; the 128x128 pair-diagonal subtiles are computed fully by BOTH
    cores, and a per-row correction column (their full row-sum) removes
    the double count at assembly time.
Per core the device emits:
  - rows_out [128,24] f32: per-(m-tile, slice) row sums of E (ACT accum),
  - cols_out [8,512]  f32: column sums of E over the core's rows for local
    columns 1024..5119 (PE ones-matmul into one PSUM bank, chunk k in
    PSUM partition k, accumulated across m-tiles),
  - g_out [128,48] f32: per-m-tile diagonals of B0..B4 (the positive-pair
    exp values + the self term) and the B4 double-count correction.
The host (the gather/unshard step) sums the row/col contributions into
F_i = sum_j E_ij, forms A_i = F_i - sum_b exp(pos), and reduces the
final scalar loss in float64: mean(log(exp(p)+A) - p).  This is the
all-reduce of the sharded partial sums; an on-device NRT collective
costs ~15-28us constant overhead, more than the entire saving.

Engine budget per core: ACT exps 128x37376 elements in 24 slices
(~49us, the bottleneck), PE ~28us (S matmuls + colsum matmuls), DVE
~10us (diag extracts + corr), one Pool copy, DMA 2MB in / 44KB out.
"""

import sys

import numpy as np

if "/opt/trn_rl_repo" not in sys.path:
    sys.path.insert(0, "/opt/trn_rl_repo")

N = 8
BS = 1024
D = 128
K = N * BS          # 8192
NCORES = 8
TEMP_SCALE = 2.0    # 1 / temperature

_NC_CACHE = {}


def _slice_sizes(m):
    """Per-m-tile activation slice widths over the 4096+w column stream."""
    w = 1024 - 128 * m
    total = 4096 + w
    if m % 2 == 0:
        sl = [2048, 1536, total - 3584]
    else:
        sl = [1536, 2048, total - 3584]
    assert sl[2] >= 128 and sl[2] <= 2048
    return sl


def _build_nc():
    import concourse.bacc as bacc
    import concourse.tile as tile
    from concourse import mybir

    f32 = mybir.dt.float32
    bf16 = mybir.dt.bfloat16
    AX = mybir.AxisListType
    OP = mybir.AluOpType
    AF = mybir.ActivationFunctionType

    nc = bacc.Bacc("TRN2", target_bir_lowering=False, debug=False)
    zt = nc.declare_dram_parameter("zt", [128, K], bf16, isOutput=False)
    ident = nc.declare_dram_parameter("ident", [128, 128], bf16, isOutput=False)
    # ind[:, 8k:8k+8] is the [128,8] indicator matrix with column k all-ones:
    # used as matmul lhsT it lands chunk k's column sums in PSUM partition k
    # (and adds zeros to the other partitions of the shared strip tile).
    ind = nc.declare_dram_parameter("ind", [128, 64], bf16, isOutput=False)
    rows_out = nc.declare_dram_parameter("rows_out", [128, 24], f32, isOutput=True)
    g_out = nc.declare_dram_parameter("g_out", [128, 48], f32, isOutput=True)
    cols_out = nc.declare_dram_parameter("cols_out", [8, 512], f32, isOutput=True)

    with tile.TileContext(nc) as tc:
        with (
            tc.tile_pool(name="persist", bufs=1) as P,
            tc.tile_pool(name="work", bufs=2) as W,
            tc.tile_pool(name="pmm", bufs=1, space="PSUM") as PM,
        ):
            # persistent SBUF.  zt lands in four 2048-col chunks issued from
            # the SP queue (gpsimd DMA triggers cost ~650ns of Q7 descriptor
            # generation EACH and serialize); the first chunk is all the
            # first m-tile needs, so compute starts ~2us in.
            idsb = P.tile([128, 128], bf16, tag="idsb")
            nc.sync.dma_start(out=idsb[:], in_=ident[:, :])
            indsb = P.tile([128, 64], bf16, tag="indsb")
            nc.sync.dma_start(out=indsb[:], in_=ind[:, :])
            ztg = []
            for g in range(4):
                zg = P.tile([128, 2048], bf16, tag=f"ztg{g}")
                ztg.append(zg)
                nc.sync.dma_start(out=zg[:], in_=zt[:, g * 2048 : (g + 1) * 2048])
            acc = P.tile([128, 72], f32, tag="acc")  # rows 0:24, g 24:72

            # PSUM: two S tiles (double buffer) + colsum strip bank
            sA = PM.tile([128, 2048], f32, tag="sA")
            sB = PM.tile([128, 1536], f32, tag="sB")
            CS = PM.tile([128, 512], f32, tag="cs")

            # warm the PE p-state ramp while the zt DMA is in flight
            for _ in range(20):
                nc.tensor.matmul(
                    sB[:, 0:128], idsb[:], idsb[:], start=True, stop=True
                )

            def rhs_ap(c0, c1):
                """zt SBUF AP for local column range [c0, c1) (single chunk)."""
                g = c0 // 2048
                assert (c1 - 1) // 2048 == g
                return ztg[g][:, c0 - g * 2048 : c1 - g * 2048]

            def emit_colsums(m, E):
                # colsum matmuls: local cols 1024..4096+w, chunk k of 512 cols
                # accumulated across m into PSUM partition k of CS via the
                # indicator lhsT (adds zeros to the other partitions)
                for k in range(8):
                    c0 = 1024 + 512 * k
                    c1 = c0 + 512
                    if k < 6:
                        j0, j1 = 0, 512
                        st0 = c0
                    else:
                        # B4 chunks: valid cols are >= 4096+128m
                        lo = max(c0, 4096 + 128 * m)
                        if lo >= c1:
                            continue
                        j0, j1 = lo - c0, 512
                        st0 = lo - 128 * m  # stream position
                    nc.tensor.matmul(
                        CS[0:8, j0:j1],
                        indsb[:, 8 * k : 8 * k + 8],
                        E[:, st0 : st0 + (j1 - j0)],
                        start=(m == 0 and k == 0),
                        stop=(m == 7 and k == 7),
                        skip_group_check=True,
                    )

            pending = None  # (m, E) whose colsums are issued one m-tile late
            for m in range(8):
                w = 1024 - 128 * m
                lhsT = ztg[0][:, m * 128 : (m + 1) * 128]
                E = W.tile([128, 5120], bf16, tag="E", bufs=3)
                sl = _slice_sizes(m)
                so = 0  # stream offset
                for si, L in enumerate(sl):
                    ps = sA if (si + m) % 2 == 0 else sB
                    # fill PSUM with S via 512-col matmul chunks
                    q0 = 0
                    while q0 < L:
                        qw = min(512, L - q0)
                        s_pos = so + q0
                        # local col of stream position
                        if s_pos < 4096:
                            c0 = s_pos
                        else:
                            c0 = s_pos + 128 * m
                        nc.tensor.matmul(
                            ps[:, q0 : q0 + qw],
                            lhsT,
                            rhs_ap(c0, c0 + qw),
                            start=True,
                            stop=True,
                        )
                        q0 += qw
                    nc.scalar.activation(
                        out=E[:, so : so + L],
                        in_=ps[:, 0:L],
                        func=AF.Exp,
                        scale=TEMP_SCALE,
                        accum_out=acc[:, 3 * m + si : 3 * m + si + 1],
                    )
                    so += L

                # diag extracts: positives + self term (stream pos k*1024+m*128
                # for k<4; B4 diag subtile sits at stream 4096)
                for k in range(5):
                    cs0 = k * 1024 + m * 128 if k < 4 else 4096
                    gc = 24 + 6 * m + k
                    scr = W.tile([128, 128], bf16, tag="scr")
                    nc.vector.scalar_tensor_tensor(
                        out=scr[:],
                        in0=E[:, cs0 : cs0 + 128],
                        scalar=1.0,
                        in1=idsb[:],
                        op0=OP.mult,
                        op1=OP.mult,
                        accum_out=acc[:, gc : gc + 1],
                    )
                # corr: full row-sum of the B4 pair-diagonal subtile
                nc.vector.tensor_reduce(
                    out=acc[:, 24 + 6 * m + 5 : 24 + 6 * m + 6],
                    in_=E[:, 4096 : 4096 + 128],
                    axis=AX.X,
                    op=OP.add,
                )

                # previous m-tile's colsums go on the PE queue here, so they
                # execute in the shadow of this m-tile's exps instead of
                # delaying the next m-tile's PSUM fills
                if pending is not None:
                    emit_colsums(*pending)
                pending = (m, E)
            emit_colsums(*pending)

            # drain CS -> SBUF -> DRAM; ship accumulators
            cs_sb = P.tile([8, 512], f32, tag="cs_sb")
            nc.vector.tensor_copy(out=cs_sb[:], in_=CS[0:8, :])
            nc.sync.dma_start(out=rows_out[:, :], in_=acc[:, 0:24])
            nc.sync.dma_start(out=g_out[:, :], in_=acc[:, 24:72])
            nc.sync.dma_start(out=cols_out[:, :], in_=cs_sb[:])

    nc.compile()
    return nc


def _get_nc():
    if "nc" not in _NC_CACHE:
        _NC_CACHE["nc"] = _build_nc()
    return _NC_CACHE["nc"]


def _host_prep(sim):
    import ml_dtypes

    x = np.asarray(sim, dtype=np.float64).reshape(K, D)
    z = (x / np.maximum(np.linalg.norm(x, axis=1, keepdims=True), 1e-8)).astype(
        np.float32
    )
    ident = np.eye(128, dtype=ml_dtypes.bfloat16)
    ind = np.zeros((128, 64), dtype=ml_dtypes.bfloat16)
    for k in range(8):
        ind[:, 8 * k + k] = 1
    in_maps = []
    for c in range(NCORES):
        ztc = np.ascontiguousarray(
            np.roll(z, -c * BS, axis=0).T.astype(ml_dtypes.bfloat16)
        )
        in_maps.append({"zt": ztc, "ident": ident, "ind": ind})
    return in_maps


def _assemble(results):
    """Gather/unshard: combine per-core partial sums into the scalar loss."""
    F = np.zeros(K, np.float64)
    G = np.zeros((K, 8), np.float64)
    li = np.arange(128)
    for c in range(NCORES):
        rows = np.asarray(results[c]["rows_out"], np.float64)
        g = np.asarray(results[c]["g_out"], np.float64)
        cols = np.asarray(results[c]["cols_out"], np.float64)
        for m in range(8):
            gr = c * BS + m * 128 + li  # global rows
            F[gr] += rows[:, 3 * m : 3 * m + 3].sum(1)
            F[gr] -= g[:, 6 * m + 5]  # B4 diag-subtile double count
            for k in range(5):
                bc = (c + k) % 8
                G[gr, bc] = g[:, 6 * m + k]
                G[bc * BS + m * 128 + li, c] = g[:, 6 * m + k]
        for k in range(8):
            gcols = (c * BS + 1024 + 512 * k + np.arange(512)) % K
            F[gcols] += cols[k]
    P = G.sum(1)
    A = F - P
    idx = np.arange(K)
    mask = np.ones((K, 8), bool)
    mask[idx, idx // BS] = False
    Epos = G[mask].reshape(K, 7)
    L = np.log(Epos + A[:, None]) - np.log(Epos)
    return np.float32(L.sum() / (K * 7))


def kernel(sim: np.ndarray, _want_results: bool = False, _trace: bool = False):
    in_maps = _host_prep(sim)
    nc = _get_nc()
    from concourse.bass_utils import run_bass_kernel_spmd

    res = run_bass_kernel_spmd(nc, in_maps, list(range(NCORES)), trace=_trace)
    loss = _assemble(res.results)
    if _want_results:
        return loss, res
    return loss


if __name__ == "__main__":
    nc = _build_nc()
    print("build OK")


# revision 16
# speedup vs baseline: 1.1502x; 1.0537x over previous
"""Trainium2 Bass kernel for DisentangleStaticNoiseLoss (NT-Xent style loss).

Math (matches the jax reference):
    x   : [K=8192, D=128] stacked embeddings (N=8 blocks of BS=1024)
    z   : row-normalized x;  S = (z @ z.T) / 0.5;  E = exp(S)
    row i (block b, sample r): positives = S[i, r + b'*BS] for b' != b,
    negatives = all j with j % BS != r.
    loss = mean over (i, pos) of [log(exp(pos) + sum_neg exp(neg)) - pos]

Sharding (data-parallel over rows, symmetric halving of the exp work):
E is symmetric, so the K x K matrix is computed only once per unordered
block pair instead of twice.  Each core receives the FULL z rotated so its
own 1024 rows come first (host-side np.roll), making the SPMD program
identical on every core.  Core c computes, for its rows, the local column
blocks B0..B3 fully plus the upper-triangular half of B4 (ragged by
128-row m-tile).  Block-pair coverage over all 8 cores:
  - difference d=1,2,3 pairs: covered once via B1..B3,
  - d=5,6,7 pairs: same unordered pairs, covered from the other side,
  - d=4 pairs: each of the two cores computes one triangle of the pair
    block; the 128x128 pair-diagonal subtiles are computed fully by BOTH
    cores, and a per-row correction column (their full row-sum) removes
    the double count at assembly time.
Per core the device emits:
  - rows_out [128,24] f32: per-(m-tile, slice) row sums of E (ACT accum),
  - cols_out [8,512]  f32: column sums of E over the core's rows for local
    columns 1024..5119 (PE ones-matmul into one PSUM bank, chunk k in
    PSUM partition k, accumulated across m-tiles),
  - g_out [128,48] f32: per-m-tile diagonals of B0..B4 (the positive-pair
    exp values + the self term) and the B4 double-count correction.
The host (the gather/unshard step) sums the row/col contributions into
F_i = sum_j E_ij, forms A_i = F_i - sum_b exp(pos), and reduces the
final scalar loss in float64: mean(log(exp(p)+A) - p).  This is the
all-reduce of the sharded partial sums; an on-device NRT collective
costs ~15-28us constant overhead, more than the entire saving.

Engine budget per core: ACT exps 128x37376 elements in 24 slices
(~49us, the bottleneck), PE ~28us (S matmuls + colsum matmuls), DVE
~10us (diag extracts + corr), one Pool copy, DMA 2MB in / 44KB out.
"""

import sys

import numpy as np

if "/opt/trn_rl_repo" not in sys.path:
    sys.path.insert(0, "/opt/trn_rl_repo")

N = 8
BS = 1024
D = 128
K = N * BS          # 8192
NCORES = 8
TEMP_SCALE = 2.0    # 1 / temperature

_NC_CACHE = {}


def _slice_sizes(m):
    """Per-m-tile activation slice widths over the 4096+w column stream."""
    w = 1024 - 128 * m
    total = 4096 + w
    if m % 2 == 0:
        sl = [2048, 1536, total - 3584]
    else:
        sl = [1536, 2048, total - 3584]
    assert sl[2] >= 128 and sl[2] <= 2048
    return sl


def _build_nc():
    import concourse.bacc as bacc
    import concourse.tile as tile
    from concourse import mybir

    f32 = mybir.dt.float32
    bf16 = mybir.dt.bfloat16
    AX = mybir.AxisListType
    OP = mybir.AluOpType
    AF = mybir.ActivationFunctionType

    nc = bacc.Bacc("TRN2", target_bir_lowering=False, debug=False)
    zt = nc.declare_dram_parameter("zt", [128, K], bf16, isOutput=False)
    ident = nc.declare_dram_parameter("ident", [128, 128], bf16, isOutput=False)
    # ind[:, 8k:8k+8] is the [128,8] indicator matrix with column k all-ones:
    # used as matmul lhsT it lands chunk k's column sums in PSUM partition k
    # (and adds zeros to the other partitions of the shared strip tile).
    ind = nc.declare_dram_parameter("ind", [128, 64], bf16, isOutput=False)
    rows_out = nc.declare_dram_parameter("rows_out", [128, 24], f32, isOutput=True)
    g_out = nc.declare_dram_parameter("g_out", [128, 48], f32, isOutput=True)
    cols_out = nc.declare_dram_parameter("cols_out", [8, 512], f32, isOutput=True)

    with tile.TileContext(nc) as tc:
        with (
            tc.tile_pool(name="persist", bufs=1) as P,
            tc.tile_pool(name="work", bufs=2) as W,
            tc.tile_pool(name="pmm", bufs=1, space="PSUM") as PM,
        ):
            # persistent SBUF.  zt lands in four 2048-col chunks issued from
            # the SP queue (gpsimd DMA triggers cost ~650ns of Q7 descriptor
            # generation EACH and serialize); the first chunk is all the
            # first m-tile needs, so compute starts as soon as it lands.
            ztg = [
                P.tile([128, 2048], bf16, tag=f"ztg{g}", name=f"ztg{g}")
                for g in range(4)
            ]
            idsb = P.tile([128, 128], bf16, tag="idsb")
            indsb = P.tile([128, 64], bf16, tag="indsb")
            for g in range(2):
                nc.sync.dma_start(out=ztg[g][:], in_=zt[:, g * 2048 : (g + 1) * 2048])
            nc.sync.dma_start(out=idsb[:], in_=ident[:, :])
            nc.sync.dma_start(out=indsb[:], in_=ind[:, :])
            for g in range(2, 4):
                nc.sync.dma_start(out=ztg[g][:], in_=zt[:, g * 2048 : (g + 1) * 2048])
            acc = P.tile([128, 72], f32, tag="acc")  # rows 0:24, g 24:72

            # PSUM: two S tiles (double buffer) + colsum strip bank
            sA = PM.tile([128, 2048], f32, tag="sA")
            sB = PM.tile([128, 1536], f32, tag="sB")
            CS = PM.tile([128, 512], f32, tag="cs")

            # warm the PE p-state ramp while the zt DMA is in flight (the
            # ramp needs ~3us of continuous execution to reach full clock)
            wtile = P.tile([128, 512], bf16, tag="wtile")
            nc.vector.memset(wtile[:], 0.03)
            for _ in range(8):
                nc.tensor.matmul(
                    sB[:, 0:512], wtile[:, 0:128], wtile[:], start=True, stop=True
                )

            def rhs_ap(c0, c1):
                """zt SBUF AP for local column range [c0, c1) (single chunk)."""
                g = c0 // 2048
                assert (c1 - 1) // 2048 == g
                return ztg[g][:, c0 - g * 2048 : c1 - g * 2048]

            def emit_colsums(m, E, win=None):
                # colsum matmuls: local cols 1024..4096+w, chunk k of 512 cols
                # accumulated across m into PSUM partition k of CS via the
                # indicator lhsT (adds zeros to the other partitions).  win
                # restricts to chunks within a stream window (m=7 per-slice).
                for k in range(8):
                    c0 = 1024 + 512 * k
                    c1 = c0 + 512
                    if k < 6:
                        j0, j1 = 0, 512
                        st0 = c0
                    else:
                        # B4 chunks: valid cols are >= 4096+128m
                        lo = max(c0, 4096 + 128 * m)
                        if lo >= c1:
                            continue
                        j0, j1 = lo - c0, 512
                        st0 = lo - 128 * m  # stream position
                    if win is not None and not (win[0] <= st0 and st0 + (j1 - j0) <= win[1]):
                        continue
                    nc.tensor.matmul(
                        CS[0:8, j0:j1],
                        indsb[:, 8 * k : 8 * k + 8],
                        E[:, st0 : st0 + (j1 - j0)],
                        start=(m == 0 and k == 0),
                        stop=(m == 7 and k == 7),
                        skip_group_check=True,
                    )

            def emit_extract(m, E, k):
                # diag extract: positives + self term (stream pos k*1024+m*128
                # for k<4; B4 pair-diag subtile sits at stream 4096)
                cs0 = k * 1024 + m * 128 if k < 4 else 4096
                gc = 24 + 6 * m + k
                scr = W.tile([128, 128], bf16, tag="scr")
                nc.vector.scalar_tensor_tensor(
                    out=scr[:],
                    in0=E[:, cs0 : cs0 + 128],
                    scalar=1.0,
                    in1=idsb[:],
                    op0=OP.mult,
                    op1=OP.mult,
                    accum_out=acc[:, gc : gc + 1],
                )

            def emit_corr(m, E):
                # corr: full row-sum of the B4 pair-diagonal subtile
                nc.vector.tensor_reduce(
                    out=acc[:, 24 + 6 * m + 5 : 24 + 6 * m + 6],
                    in_=E[:, 4096 : 4096 + 128],
                    axis=AX.X,
                    op=OP.add,
                )

            pending = None  # (m, E) whose colsums are issued one m-tile late
            for m in range(8):
                w = 1024 - 128 * m
                lhsT = ztg[0][:, m * 128 : (m + 1) * 128]
                E = W.tile([128, 5120], bf16, tag="E", bufs=3)
                sl = _slice_sizes(m)
                act_si = 0 if m % 2 == 0 else 1  # largest slice keeps ACT accum
                so = 0  # stream offset
                for si, L in enumerate(sl):
                    ps = sA if (si + m) % 2 == 0 else sB
                    # fill PSUM with S via 512-col matmul chunks
                    q0 = 0
                    while q0 < L:
                        qw = min(512, L - q0)
                        s_pos = so + q0
                        # local col of stream position
                        if s_pos < 4096:
                            c0 = s_pos
                        else:
                            c0 = s_pos + 128 * m
                        nc.tensor.matmul(
                            ps[:, q0 : q0 + qw],
                            lhsT,
                            rhs_ap(c0, c0 + qw),
                            start=True,
                            stop=True,
                        )
                        q0 += qw
                    nc.scalar.activation(
                        out=E[:, so : so + L],
                        in_=ps[:, 0:L],
                        func=AF.Exp,
                        scale=TEMP_SCALE,
                        **(
                            {"accum_out": acc[:, 3 * m + si : 3 * m + si + 1]}
                            if si == act_si
                            else {}
                        ),
                    )
                    if si != act_si:
                        # row sums for the smaller slices ride on DVE instead
                        # of costing ACT an accumulator read
                        nc.vector.tensor_reduce(
                            out=acc[:, 3 * m + si : 3 * m + si + 1],
                            in_=E[:, so : so + L],
                            axis=AX.X,
                            op=OP.add,
                        )
                    if m == 7:
                        # final m-tile: drain everything per-slice to shrink
                        # the serial tail; m=6's colsums slot in right after
                        # the first fill so they stay off the tail
                        if si == 0 and pending is not None:
                            emit_colsums(*pending)
                            pending = None
                        for k in range(5):
                            cs0 = k * 1024 + m * 128 if k < 4 else 4096
                            if so <= cs0 and cs0 + 128 <= so + L:
                                emit_extract(m, E, k)
                        if so <= 4096 < so + L:
                            emit_corr(m, E)
                        emit_colsums(m, E, win=(so, so + L))
                    so += L

                if m < 7:
                    for k in range(5):
                        emit_extract(m, E, k)
                    emit_corr(m, E)
                    # previous m-tile's colsums go on the PE queue here, so
                    # they execute in the shadow of this m-tile's exps instead
                    # of delaying the next m-tile's PSUM fills
                    if pending is not None:
                        emit_colsums(*pending)
                    pending = (m, E)

            # drain CS -> SBUF -> DRAM; ship accumulators
            cs_sb = P.tile([8, 512], f32, tag="cs_sb")
            nc.vector.tensor_copy(out=cs_sb[:], in_=CS[0:8, :])
            nc.sync.dma_start(out=rows_out[:, :], in_=acc[:, 0:24])
            nc.sync.dma_start(out=g_out[:, :], in_=acc[:, 24:72])
            nc.sync.dma_start(out=cols_out[:, :], in_=cs_sb[:])

    nc.compile()
    return nc


def _get_nc():
    if "nc" not in _NC_CACHE:
        _NC_CACHE["nc"] = _build_nc()
    return _NC_CACHE["nc"]


def _host_prep(sim):
    import ml_dtypes

    x = np.asarray(sim, dtype=np.float64).reshape(K, D)
    z = (x / np.maximum(np.linalg.norm(x, axis=1, keepdims=True), 1e-8)).astype(
        np.float32
    )
    ident = np.eye(128, dtype=ml_dtypes.bfloat16)
    ind = np.zeros((128, 64), dtype=ml_dtypes.bfloat16)
    for k in range(8):
        ind[:, 8 * k + k] = 1
    in_maps = []
    for c in range(NCORES):
        ztc = np.ascontiguousarray(
            np.roll(z, -c * BS, axis=0).T.astype(ml_dtypes.bfloat16)
        )
        in_maps.append({"zt": ztc, "ident": ident, "ind": ind})
    return in_maps


def _assemble(results):
    """Gather/unshard: combine per-core partial sums into the scalar loss."""
    F = np.zeros(K, np.float64)
    G = np.zeros((K, 8), np.float64)
    li = np.arange(128)
    for c in range(NCORES):
        rows = np.asarray(results[c]["rows_out"], np.float64)
        g = np.asarray(results[c]["g_out"], np.float64)
        cols = np.asarray(results[c]["cols_out"], np.float64)
        for m in range(8):
            gr = c * BS + m * 128 + li  # global rows
            F[gr] += rows[:, 3 * m : 3 * m + 3].sum(1)
            F[gr] -= g[:, 6 * m + 5]  # B4 diag-subtile double count
            for k in range(5):
                bc = (c + k) % 8
                G[gr, bc] = g[:, 6 * m + k]
                G[bc * BS + m * 128 + li, c] = g[:, 6 * m + k]
        for k in range(8):
            gcols = (c * BS + 1024 + 512 * k + np.arange(512)) % K
            F[gcols] += cols[k]
    P = G.sum(1)
    A = F - P
    idx = np.arange(K)
    mask = np.ones((K, 8), bool)
    mask[idx, idx // BS] = False
    Epos = G[mask].reshape(K, 7)
    L = np.log(Epos + A[:, None]) - np.log(Epos)
    return np.float32(L.sum() / (K * 7))


def kernel(sim: np.ndarray, _want_results: bool = False, _trace: bool = False):
    in_maps = _host_prep(sim)
    nc = _get_nc()
    from concourse.bass_utils import run_bass_kernel_spmd

    res = run_bass_kernel_spmd(nc, in_maps, list(range(NCORES)), trace=_trace)
    loss = _assemble(res.results)
    if _want_results:
        return loss, res
    return loss


if __name__ == "__main__":
    nc = _build_nc()
    print("build OK")
